# revision 1
# baseline (speedup 1.0000x reference)
"""EMAttention2d (vq_codebook) Trainium2 kernel — fp16 + DMA-transpose design.

Data parallel over batch: 16 images -> 8 cores x 2 images. BN batch stats
cross-core reduced with a tiny AllReduce.

Numerics (validated vs reference in numpy, rel err ~6e-3 < 2e-2 tol):
  - fp16 everywhere except: logit/mu-update accumulation in fp32 PSUM,
    exp output in bf16 (exp(A) can reach ~2e9, overflows fp16),
    softmax sums / BN statistics in fp32.
  - x is pre-cast to fp16 on the host (free), output is stored fp16 and
    upcast to fp32 on the host.

Math per image, X = x[b] (C, N) fp16:
    mut = Ws^T mu           (C, K)  [stem folded into codebook]
    mub = mu^T bs           (K,)
    repeat 3x:
      A^T  = mut^T X + mub  (K, N) fp32 psum
      E^T  = exp(A^T)       (K, N) bf16
      e3   = dma-transpose(E^T)  -> (n-part, K)
      z    = e3 / sum_k e3  fp16  [proper softmax]
      G    = z^T X^T        (K, C) psum   [X^T via dma-transpose, fp16]
      s_k  = z^T 1          (K,)
      mur  = G Ws^T + s_k bs^T    (K, C) psum
      muT  = mur * rsqrt(C*(var+mean^2))  [row L2 norm via bn_stats]
      mu_nat = dma-transpose(muT); mut = Ws^T mu_nat (PE); mub = mu_nat^T bs
    zz   = dma-transpose(z)  [z^T tiles for y2]
    y2   = muT^T z^T        (C, N)
    r    = relu(y2) fp16;  h = Hw r  (C, N) -> fp16 SBUF resident
  BN over batch of h: per-tile bn_stats -> bn_aggr -> (mean, E[x^2]) pack,
  AllReduce(add), then out = relu(h*a + b2 + x), a = gamma*rstd,
  b2 = beta - mean*a.  Stored fp16, host astype(fp32).

Engines: PE runs only matmuls; every transpose is a DMA-engine xbar
transpose; z normalize multiply on gpsimd; exp/relu/copies split
Act/DVE; BN stats via DVE bn_stats.
"""

import sys

for _p in ("/opt/trn_rl_repo",):
    if _p not in sys.path:
        sys.path.insert(0, _p)

import numpy as np

B, C, N, K = 16, 512, 4096, 64
NCORES = 8
BPC = B // NCORES  # images per core
P = 128
OC = C // P   # 4 chunks of channels
NT = N // P   # 32 pixel tiles
NCH = N // 512  # 8 pixel chunks of 512
BN_EPS = 1e-5
NUM_ITER = 3

_cache = {}


def _build_nc(n_devices=NCORES, use_collective=True, debug_dump=False,
              bias=True):
    import concourse.bass as bass
    import concourse.mybir as mybir
    import concourse.tile as tile
    from concourse.masks import make_identity
    from contextlib import ExitStack

    dt = mybir.dt
    f32 = dt.float32
    f16 = dt.float16
    bf16 = dt.bfloat16
    AF = mybir.ActivationFunctionType
    ALU = mybir.AluOpType
    AX = mybir.AxisListType

    nc = bass.Bass("TRN2", target_bir_lowering=False, debug=False,
                   num_devices=n_devices)

    xh_d = nc.dram_tensor("xh", [BPC, C, N], f16, kind="ExternalInput").ap()
    mu_d = nc.dram_tensor("mu", [C, K], f16, kind="ExternalInput").ap()
    ws_d = nc.dram_tensor("ws", [C, C], f16, kind="ExternalInput").ap()
    wst_d = nc.dram_tensor("wst", [C, C], f16, kind="ExternalInput").ap()
    hwt_d = nc.dram_tensor("hwt", [C, C], f16, kind="ExternalInput").ap()
    bs_d = nc.dram_tensor("bs", [C, 2], f16, kind="ExternalInput").ap()
    gm_d = nc.dram_tensor("gm", [C], f32, kind="ExternalInput").ap()
    bt_d = nc.dram_tensor("bt", [C], f32, kind="ExternalInput").ap()
    onec_d = nc.dram_tensor("onec", [P], f16, kind="ExternalInput").ap()
    out_d = nc.dram_tensor("out", [BPC, C, N], f16, kind="ExternalOutput").ap()
    st_in_d = nc.dram_tensor("stats_in", [P, 2 * OC], f32).ap()
    st_out_d = nc.dram_tensor("stats_out", [P, 2 * OC], f32,
                              addr_space="Shared").ap()
    if debug_dump:
        dz_d = nc.dram_tensor("dz", [BPC, P, NT, K], f16,
                              kind="ExternalOutput").ap()
        dmu_d = nc.dram_tensor("dmu", [BPC, K, C], f16,
                               kind="ExternalOutput").ap()
        dh_d = nc.dram_tensor("dh", [BPC, P, OC, N], f16,
                              kind="ExternalOutput").ap()
        dmut_d = nc.dram_tensor("dmut", [P, OC, K], f16,
                                kind="ExternalOutput").ap()
        dst_d = nc.dram_tensor("dst", [P, 6 * OC], f32,
                               kind="ExternalOutput").ap()

    with tile.TileContext(nc) as tc, ExitStack() as ctx:
        consts = ctx.enter_context(tc.tile_pool(name="consts", bufs=1))
        statp = ctx.enter_context(tc.tile_pool(name="statp", bufs=1))
        zpool = ctx.enter_context(tc.tile_pool(name="zpool", bufs=1))
        mutp = ctx.enter_context(tc.tile_pool(name="mutp", bufs=1))
        xpool = ctx.enter_context(tc.tile_pool(name="xpool", bufs=1))
        hpool = ctx.enter_context(tc.tile_pool(name="hpool", bufs=1))
        psum = ctx.enter_context(tc.tile_pool(name="psum", bufs=1,
                                              space="PSUM"))
        # EM/L3-scoped pools (closed before the final pass so fstage can
        # use their space)
        emctx = ExitStack()
        xtpool = emctx.enter_context(tc.tile_pool(name="xtpool", bufs=1))
        etp = emctx.enter_context(tc.tile_pool(name="etp", bufs=2))
        e3p = emctx.enter_context(tc.tile_pool(name="e3p", bufs=3))
        smalls = emctx.enter_context(tc.tile_pool(name="smalls", bufs=2))
        mutsp = emctx.enter_context(tc.tile_pool(name="mutsp", bufs=3))
        ry2p = emctx.enter_context(tc.tile_pool(name="ry2p", bufs=1))

        def ps(nm, tag):
            return psum.tile([P, 512], f32, tag=tag, name=nm)

        # ---- constants (wst/hwt loads deferred to when first needed) ----
        ws_sb = consts.tile([P, OC, C], f16)     # Ws rows (o_part, oc, c)
        nc.sync.dma_start(ws_sb[:], ws_d.rearrange("(t p) c -> p t c", p=P))
        mu0_sb = consts.tile([P, OC, K], f16)    # mu natural (c_part, oc, k)
        nc.sync.dma_start(mu0_sb[:], mu_d.rearrange("(t p) k -> p t k", p=P))
        b_sb = consts.tile([P, OC, 2], f16)
        nc.sync.dma_start(b_sb[:],
                          bs_d.rearrange("(t p) two -> p t two", p=P))
        bs_row = consts.tile([1, C], f16)
        nc.sync.dma_start(bs_row[:], bs_d[:, 0][None, :])
        ones_col = consts.tile([P, 1], f16)
        nc.sync.dma_start(ones_col[:], onec_d[:, None])
        wst_sb = consts.tile([P, OC, C], f16)    # Ws^T rows (ci_part, cc, o)
        hwt_sb = consts.tile([P, OC, C], f16)    # Hw^T rows (ci_part, cc, o)
        gm_sb = consts.tile([P, OC], f32)
        bt_sb = consts.tile([P, OC], f32)
        eps_sb = consts.tile([P, 1], f32)
        nc.vector.memset(eps_sb[:], BN_EPS)
        id16 = consts.tile([K, K], f16)
        make_identity(nc, id16[:])

        hstat = statp.tile([P, OC, BPC * NCH, 6], f32)
        x_sb = [xpool.tile([P, OC, N], f16, tag=f"x{b}", name=f"x{b}")
                for b in range(BPC)]
        h_sb = [hpool.tile([P, OC, N], f16, tag=f"h{b}", name=f"h{b}")
                for b in range(BPC)]
        # X^T halves: [P, OC, 16, P]; half hf covers pixel tiles 16*hf..+15
        xt_sb = {}

        z_of = [None] * BPC
        zz_of = [None] * BPC
        muT_of = [None] * BPC

        def load_xq(b, q, eng=None):
            eng = eng or nc.sync
            for cc in range(OC):
                eng.dma_start(
                    x_sb[b][:, cc, q * 1024:(q + 1) * 1024],
                    xh_d[b, cc * P:(cc + 1) * P, q * 1024:(q + 1) * 1024])

        def xt_tile(b, hf):
            t = xtpool.tile([P, OC, 16, P], f16, tag=f"xt{hf}",
                            name=f"xt{b}_{hf}")
            xt_sb[(b, hf)] = t
            return t

        def xt_piece(b, hf, cc, uu, eng=None):
            # one [128, 1024] -> [128, 8, 128] xbar transpose (~0.9us)
            eng = eng or nc.sync
            t = xt_sb[(b, hf)]
            eng.dma_start_transpose(
                t[:, cc, uu * 8:(uu + 1) * 8, :],
                x_sb[b][:, cc, hf * 2048 + uu * 1024:
                        hf * 2048 + (uu + 1) * 1024])

        def mut_from(b_tag, mun):
            """mut = Ws^T mu ; mub = mu^T bs from natural-layout mu."""
            mut_ps = ps("mutmm" + b_tag, "MM")
            m3 = mut_ps[:, :OC * K].rearrange("p (j k) -> p j k", k=K)
            for cc in range(OC):
                for oc in range(OC):
                    nc.tensor.matmul(m3[:, cc, :],
                                     ws_sb[:, oc, cc * P:(cc + 1) * P],
                                     mun[:, oc, :],
                                     start=(oc == 0), stop=(oc == OC - 1))
            mut_sb = mutsp.tile([P, OC, K], f16, tag="mut",
                                name="mut" + b_tag)
            nc.vector.tensor_copy(mut_sb[:], m3[:])
            if not bias:
                return mut_sb, None
            mub_ps = ps("mubmm" + b_tag, "MM")
            for oc in range(OC):
                nc.tensor.matmul(mub_ps[:K, :2], mun[:, oc, :],
                                 b_sb[:, oc, :],
                                 start=(oc == 0), stop=(oc == OC - 1))
            mub_sb = mutsp.tile([K, 1], f32, tag="mub",
                                name="mub" + b_tag)
            nc.scalar.copy(mub_sb[:], mub_ps[:K, :1])
            return mut_sb, mub_sb

        def em_chunk(b, it, ch, mut_sb, mub_sb, tagp="A"):
            """A^T chunk -> exp -> transpose -> z.  No G here."""
            a_ps = ps(f"A{b}_{it}_{ch}", "%s%d" % (tagp, ch % 2))
            for cc in range(OC):
                nc.tensor.matmul(a_ps[:K, :], mut_sb[:, cc, :],
                                 x_sb[b][:, cc, ch * 512:(ch + 1) * 512],
                                 start=(cc == 0), stop=(cc == OC - 1))
            et = etp.tile([K, 512], bf16, tag="et")
            if mub_sb is not None:
                nc.scalar.activation(et[:], a_ps[:K, :], AF.Exp,
                                     bias=mub_sb[:], scale=1.0)
            else:
                nc.scalar.activation(et[:], a_ps[:K, :], AF.Exp)
            e3 = e3p.tile([P, 4, K], bf16, tag="e3")
            nc.sync.dma_start_transpose(e3[:], et[:])
            s4 = smalls.tile([P, 4], f32, tag="s4")
            nc.vector.tensor_reduce(s4[:], e3[:], axis=AX.X, op=ALU.add)
            nc.vector.reciprocal(s4[:], s4[:])
            nc.gpsimd.tensor_tensor(
                z_of[b][:, ch * 4:(ch + 1) * 4, :], e3[:],
                s4[:, :, None].to_broadcast((P, 4, K)), ALU.mult)

        def g_chunk(b, it, ch, G_ps, sk_ps):
            for j in range(4):
                tt = ch * 4 + j
                nc.tensor.matmul(G_ps[:K, :], z_of[b][:, tt, :],
                                 xt_sb[(b, tt // 16)][:, :, tt % 16, :],
                                 start=(tt == 0), stop=(tt == NT - 1))
            if bias:
                nc.tensor.matmul(sk_ps[:1, :4 * K], ones_col[:],
                                 z_of[b][:, ch * 4:(ch + 1) * 4, :],
                                 start=(ch == 0), stop=(ch == NCH - 1))

        def em_boundary(b, it, G_ps, sk_ps, fillers=None):
            """G -> mur -> normalized muT; next-iter mut/mub unless last.
            `fillers` are PE-work thunks emitted between chain stages so the
            tensor engine has something to chew on while the serial
            DVE/Act/DMA chain runs."""
            fillers = list(fillers or [])

            def fill1():
                if fillers:
                    fillers.pop(0)()

            fill1()
            if bias:
                sk_sb = mutsp.tile([1, K], f16, tag="sk")
                with nc.allow_low_precision(
                        reason="softmax col sums, fp16 ok"):
                    nc.vector.tensor_reduce(
                        sk_sb[:],
                        sk_ps[:1, :4 * K].rearrange("p (f k) -> p k f", k=K),
                        axis=AX.X, op=ALU.add)
            g_sb = smalls.tile([K, C], f16, tag="g")
            nc.scalar.copy(g_sb[:], G_ps[:K, :])
            # gt = G^T via PE transpose (lower latency than a DMA xbar pass)
            gt_ps = psum.tile([P, OC * K], f16, tag="MM", name=f"gtp{b}{it}")
            g4 = gt_ps[:].rearrange("p (j k) -> p j k", k=K)
            for cc in range(OC):
                nc.tensor.transpose(g4[:, cc, :],
                                    g_sb[:, cc * P:(cc + 1) * P], id16[:])
            gt_sb = smalls.tile([P, OC, K], f16, tag="gt")
            nc.vector.tensor_copy(gt_sb[:], g4[:])
            mur_ps = ps(f"mur{b}_{it}", "MM")
            for cc in range(OC):
                nc.tensor.matmul(mur_ps[:K, :], gt_sb[:, cc, :],
                                 wst_sb[:, cc, :],
                                 start=(cc == 0),
                                 stop=(not bias and cc == OC - 1))
            if bias:
                nc.tensor.matmul(mur_ps[:K, :], sk_sb[:], bs_row[:],
                                 start=False, stop=True)
            fill1()
            # row norm: ||row||^2 = C*(var + mean^2) via bn_stats
            n6 = smalls.tile([K, 6], f32, tag="n6")
            nc.vector.bn_stats(n6[:], mur_ps[:K, :])
            n2 = smalls.tile([K, 2], f32, tag="n2")
            nc.vector.bn_aggr(n2[:], n6[:])
            nrm = smalls.tile([K, 1], f32, tag="nrm")
            nc.vector.tensor_tensor(nrm[:], n2[:, 0:1], n2[:, 0:1], ALU.mult)
            nc.vector.tensor_tensor(nrm[:], nrm[:], n2[:, 1:2], ALU.add)
            nc.scalar.activation(nrm[:], nrm[:], AF.Ln, scale=float(C))
            nc.scalar.activation(nrm[:], nrm[:], AF.Exp, scale=-0.5)
            muT_sb = mutp.tile([K, C], f16, tag=f"muT{b}",
                               name=f"muT{b}_{it}")
            nc.vector.tensor_scalar(muT_sb[:], mur_ps[:K, :], nrm[:],
                                    None, ALU.mult)
            muT_of[b] = muT_sb
            fill1()
            if it < NUM_ITER - 1:
                mn_ps = psum.tile([P, OC * K], f16, tag="MM",
                                  name=f"mnp{b}{it}")
                m4 = mn_ps[:].rearrange("p (j k) -> p j k", k=K)
                for cc in range(OC):
                    nc.tensor.transpose(
                        m4[:, cc, :],
                        muT_sb[:, cc * P:(cc + 1) * P], id16[:])
                mun = mutsp.tile([P, OC, K], f16, tag="mun")
                nc.vector.tensor_copy(mun[:], m4[:])
                r = mut_from(f"{b}_{it}", mun)
                for f in fillers:
                    f()
                return r
            # last iter: make z^T tiles; the xbar transpose puts odd pixel
            # tiles in partitions 64..127, and PE matmuls cannot source a
            # nonzero base partition, so shift the upper half into its own
            # base-0 tile with a partition-offset DMA.
            # zz1 reuses z0's slot (z0 is dead once zz0 and G0 are done)
            zz = zpool.tile([P, 16, P], f16,
                            tag=("zz0" if b == 0 else "z0"), name=f"zz{b}")
            nc.sync.dma_start_transpose(
                zz[:], z_of[b][:].rearrange("p t k -> p (t k)"))
            # zzo1 reuses z1's slot (z1 is dead once zz1 is transposed)
            zzo = zpool.tile([K, 16, P], f16,
                             tag=("zzo0" if b == 0 else "z1"),
                             name=f"zzo{b}")
            nc.sync.dma_start(zzo[:], zz[K:2 * K, :, :])
            zz_of[b] = (zz, zzo)
            for f in fillers:
                f()
            return None, None

        def l3_chunk(b, ch):
            """y2 -> relu -> head conv -> h store + bn_stats, one 512 chunk"""
            muT_sb = muT_of[b]
            zz, zzo = zz_of[b]
            ry2 = ry2p.tile([P, OC, 512], f16, tag="ry2")
            for ot in range(OC):
                y2_ps = ps(f"y2_{b}_{ch}_{ot}", "Y%d" % (ot % 2))
                y2v = y2_ps[:].rearrange("p (j e q) -> p e j q", e=2, q=P)
                for e, zt in ((0, zz), (1, zzo)):
                    nc.tensor.matmul(
                        y2v[:, e, :, :],
                        muT_sb[:, ot * P:(ot + 1) * P],
                        zt[0:K, 2 * ch:2 * ch + 2, :],
                        start=True, stop=True)
                if ot % 2 == 0:
                    nc.vector.tensor_scalar(ry2[:, ot, :], y2_ps[:],
                                            0.0, None, ALU.max)
                else:
                    nc.scalar.activation(ry2[:, ot, :], y2_ps[:], AF.Relu)
            for o2 in range(OC):
                h_ps = ps(f"h{b}_{ch}_{o2}", "H%d" % (o2 % 2))
                for oc in range(OC):
                    nc.tensor.matmul(h_ps[:], hwt_sb[:, oc, o2 * P:(o2 + 1) * P],
                                     ry2[:, oc, :],
                                     start=(oc == 0), stop=(oc == OC - 1))
                dap = h_sb[b][:, o2, ch * 512:(ch + 1) * 512]
                nc.scalar.copy(dap, h_ps[:])
                nc.vector.bn_stats(hstat[:, o2, b * NCH + ch, :], dap)

        # ================= emission =================
        # phase 0: shared iter-0 codebook; x0 cols 0..1023.  Everything
        # else (rest of x0, X^T(0) pieces, wst, x1, hwt/gm/bt) trickles
        # into the EM-0 chunk loop, AFTER each chunk's z-chain transpose,
        # so the latency-critical DMAs are never queued behind bulk.
        mut_sh, mub_sh = mut_from("S", mu0_sb)
        load_xq(0, 0)
        z_of[0] = zpool.tile([P, NT, K], f16, tag="z0", name="z0")
        z_of[1] = zpool.tile([P, NT, K], f16, tag="z1", name="z1")
        xt_tile(0, 0)
        xt_tile(0, 1)

        def ldq(b, q):
            return lambda: load_xq(b, q)

        def xtp(b, hf, cc, uu):
            return lambda: xt_piece(b, hf, cc, uu)

        # bulk DMAs all go out on the Activation hwdge queue so they are
        # dispatched promptly instead of queuing behind the z-chain's
        # semaphore waits on the SP queue.
        drip_iter = [None] * NUM_ITER
        drip_iter[0] = (
            [ldq(0, 1)]
            + [xtp(0, 0, cc, 0) for cc in range(OC)]
            + [ldq(0, 2)]
            + [xtp(0, 0, cc, 1) for cc in range(OC)]
            + [lambda: nc.sync.dma_start(
                wst_sb[:], wst_d.rearrange("(t p) c -> p t c", p=P))]
            + [ldq(1, 0)]
            + [xtp(0, 1, cc, 0) for cc in range(OC)]
            + [ldq(0, 3)]
            + [xtp(0, 1, cc, 1) for cc in range(OC)]
            + [ldq(1, 1)])
        drip_iter[1] = [ldq(1, 2), ldq(1, 3),
                        lambda: nc.sync.dma_start(
                            hwt_sb[:],
                            hwt_d.rearrange("(t p) c -> p t c", p=P))]
        drip_iter[2] = [
            lambda: nc.sync.dma_start(
                gm_sb[:], gm_d.rearrange("(t p) -> p t", p=P)),
            lambda: nc.sync.dma_start(
                bt_sb[:], bt_d.rearrange("(t p) -> p t", p=P))]

        # phase 1: EM image 0; A/exp/z of image-1 iter 0 fills the serial
        # boundary chain.  G matmuls lag the A chunks by GLAG so PE never
        # blocks on the z-chain (exp -> dma transpose -> normalize).
        GLAG = 3
        fill = [(1, 0, ch) for ch in range(NCH)]  # img1 iter0 chunks
        mut_b = [mut_sh, mut_sh]
        mub_b = [mub_sh, mub_sh]

        def fill_thunk():
            b1, it1, ch1 = fill.pop(0)
            # image-1 prefill runs in the (not yet used) Y psum banks
            return lambda: em_chunk(b1, it1, ch1, mut_b[1], mub_b[1],
                                    tagp="Y")

        for it in range(NUM_ITER):
            G_ps = ps(f"G0_{it}", "G")
            sk_ps = ps(f"sk0_{it}", "MM") if bias else None
            dripq = list(drip_iter[it])
            per = max(1, (len(dripq) + NCH - 1) // NCH)
            gq = []
            for ch in range(NCH):
                em_chunk(0, it, ch, mut_b[0], mub_b[0])
                for _ in range(per):
                    if dripq:
                        dripq.pop(0)()
                gq.append(ch)
                if len(gq) > GLAG:
                    g_chunk(0, it, gq.pop(0), G_ps, sk_ps)
                if it > 0 and ch in (2, 5) and fill:
                    fill_thunk()()
            while gq:
                g_chunk(0, it, gq.pop(0), G_ps, sk_ps)
            while dripq:
                dripq.pop(0)()
            fillers = [fill_thunk() for _ in range(min(2, len(fill)))]
            m, mb = em_boundary(0, it, G_ps, sk_ps, fillers)
            if m is not None:
                mut_b[0] = m
                mub_b[0] = mb
        while fill:
            fill_thunk()()

        # phase 2: EM image 1 (iter0 A/z already done) interleaved with
        # L3 of image 0.  The X^T(1) xbar transposes (~14us of DMA) are
        # covered by emitting L3 chunks ahead of the G matmuls that need
        # them.
        l3q = list(range(NCH))  # img0 L3 chunks

        def drip(n):
            for _ in range(n):
                if l3q:
                    l3_chunk(0, l3q.pop(0))

        for hf in range(2):
            xt_tile(1, hf)
            for cc in range(OC):
                for uu in range(2):
                    xt_piece(1, hf, cc, uu)

        for it in range(NUM_ITER):
            G_ps = ps(f"G1_{it}", "G")
            sk_ps = ps(f"sk1_{it}", "MM") if bias else None
            gq = []
            for ch in range(NCH):
                if it > 0:
                    em_chunk(1, it, ch, mut_b[1], mub_b[1])
                    gq.append(ch)
                    if len(gq) > GLAG:
                        g_chunk(1, it, gq.pop(0), G_ps, sk_ps)
                else:
                    g_chunk(1, it, ch, G_ps, sk_ps)
                if ch % 3 == 2:
                    drip(1)
            while gq:
                g_chunk(1, it, gq.pop(0), G_ps, sk_ps)
            fillers = []
            if l3q:
                fillers.append(
                    lambda c=l3q.pop(0): l3_chunk(0, c))
            m, mb = em_boundary(1, it, G_ps, sk_ps, fillers)
            if m is not None:
                mut_b[1] = m
                mub_b[1] = mb
        drip(NCH)

        # phase 3: L3 image 1
        for ch in range(NCH):
            l3_chunk(1, ch)

        if debug_dump:
            for b in range(BPC):
                nc.sync.dma_start(dz_d[b], z_of[b][:])
                nc.sync.dma_start(dmu_d[b], muT_of[b][:])
                nc.sync.dma_start(dh_d[b], h_sb[b][:])
            nc.sync.dma_start(dmut_d[:], mut_b[0][:])

        # EM/L3 scratch no longer needed; reclaim for the final staging.
        emctx.close()
        fstage = ctx.enter_context(tc.tile_pool(name="fstage", bufs=5))

        # ---- BN stats: aggregate, AllReduce, affine coefficients ----
        mv = statp.tile([P, OC, 2], f32)
        for o2 in range(OC):
            nc.vector.bn_aggr(mv[:, o2, :], hstat[:, o2, :, :])
        pack = statp.tile([P, 2 * OC], f32)
        packv = pack[:].rearrange("p (o two) -> p o two", two=2)
        msq = statp.tile([P, OC], f32)
        nc.vector.tensor_tensor(msq[:], mv[:, :, 0], mv[:, :, 0], ALU.mult)
        nc.vector.tensor_copy(packv[:, :, 0], mv[:, :, 0])
        nc.vector.tensor_tensor(packv[:, :, 1], mv[:, :, 1], msq[:], ALU.add)
        nc.sync.dma_start(st_in_d[:], pack[:])
        if use_collective:
            nc.gpsimd.collective_compute(
                "AllReduce", ALU.add,
                replica_groups=[list(range(n_devices))],
                ins=[st_in_d[:]],
                outs=[st_out_d[:]],
            )
            red_src = st_out_d
        else:
            red_src = st_in_d
        red = statp.tile([P, 2 * OC], f32)
        nc.sync.dma_start(red[:], red_src[:])
        redv = red[:].rearrange("p (o two) -> p o two", two=2)
        mbar = statp.tile([P, OC], f32)
        nc.vector.tensor_scalar(mbar[:], redv[:, :, 0], 1.0 / NCORES, None,
                                ALU.mult)
        var = statp.tile([P, OC], f32)
        nc.vector.tensor_scalar(var[:], redv[:, :, 1], 1.0 / NCORES, None,
                                ALU.mult)
        nc.vector.tensor_tensor(msq[:], mbar[:], mbar[:], ALU.mult)
        nc.vector.tensor_tensor(var[:], var[:], msq[:], ALU.subtract)
        a_sb = statp.tile([P, OC], f32)
        # rstd = exp(-0.5*ln(var + eps))
        nc.scalar.activation(a_sb[:], var[:], AF.Ln, bias=eps_sb[:])
        nc.scalar.activation(a_sb[:], a_sb[:], AF.Exp, scale=-0.5)
        nc.vector.tensor_tensor(a_sb[:], a_sb[:], gm_sb[:], ALU.mult)
        b2_sb = statp.tile([P, OC], f32)
        nc.vector.tensor_tensor(b2_sb[:], mbar[:], a_sb[:], ALU.mult)
        nc.vector.tensor_tensor(b2_sb[:], bt_sb[:], b2_sb[:], ALU.subtract)
        if debug_dump:
            dstv = dst_d.rearrange("p (six o) -> p six o", o=OC)
            nc.sync.dma_start(dstv[:, 0, :], mv[:, :, 0])
            nc.sync.dma_start(dstv[:, 1, :], mv[:, :, 1])
            nc.sync.dma_start(dstv[:, 2, :], mbar[:])
            nc.sync.dma_start(dstv[:, 3, :], var[:])
            nc.sync.dma_start(dstv[:, 4, :], a_sb[:])
            nc.sync.dma_start(dstv[:, 5, :], b2_sb[:])

        # ---- final: out = relu(h*a + b2 + x), all fp16 fast ops ----
        # u = a*h + b2 (tensor_scalar, 4x fp16); w = u + x (tensor_tensor,
        # 2x); relu mostly on Act (balances DVE); store fp16.
        fi = 0
        for b in range(BPC):
            for o2 in range(OC):
                for fc in range(2):
                    sl = slice(fc * 2048, (fc + 1) * 2048)
                    u = fstage.tile([P, 2048], f16, tag="u")
                    nc.vector.tensor_scalar(
                        u[:], h_sb[b][:, o2, sl], a_sb[:, o2:o2 + 1],
                        b2_sb[:, o2:o2 + 1], ALU.mult, ALU.add)
                    nc.vector.tensor_tensor(u[:], u[:],
                                            x_sb[b][:, o2, sl], ALU.add)
                    r = fstage.tile([P, 2048], f16, tag="r")
                    if fi % 4 == 3:
                        nc.gpsimd.tensor_scalar(r[:], u[:], 0.0, None,
                                                ALU.max)
                    else:
                        nc.scalar.activation(r[:], u[:], AF.Relu)
                    fi += 1
                    nc.sync.dma_start(
                        out_d[b, o2 * P:(o2 + 1) * P, sl], r[:])

    _hoist_extra_waits(nc)
    return nc


_ENGINE_SEM_PREFIX = {
    "EngineType.PE": "PE_",
    "EngineType.Activation": "Activation_",
    "EngineType.DVE": "DVE_",
    "EngineType.Pool": "Pool_",
    "EngineType.SP": "SP_",
}


def _hoist_extra_waits(nc):
    """This walrus build rejects compute-engine instructions carrying more
    than one sync wait. Engine queues are strict FIFO, so (a) a COMPUTE
    instruction waiting on its own engine's semaphore is always already
    satisfied -> drop it; (b) any extra waits can be hoisted onto NoOp
    instructions injected just before, one wait each -- identical
    semantics.  DMA instructions dispatch at SEQ level without entering
    the engine pipeline, so their own-engine waits are real dependencies
    (e.g. a dma reading an activation's output from the same queue) and
    must be KEPT (hoisted onto a NoOp, which does block the queue)."""
    import concourse.mybir as mybir
    _DMA_INSTS = ("InstDMACopy", "InstDmaTransposeAnt", "InstTriggerDma",
                  "InstDMAGatherAnt", "InstDMAScatterAddAnt")
    nid = 0
    for blk in nc.m.functions[0].blocks:
        out = []
        changed = False
        for i in blk.instructions:
            si = getattr(i, "sync_info", None)
            eng = str(getattr(i, "engine", None))
            waits = list(si.on_wait) if si and si.on_wait else []
            if len(waits) > 1 and eng in _ENGINE_SEM_PREFIX:
                selfp = _ENGINE_SEM_PREFIX[eng]
                if type(i).__name__ not in _DMA_INSTS:
                    waits = [w for w in waits
                             if not w.ant_name.startswith(selfp)]
                for w in waits[:-1]:
                    nid += 1
                    out.append(mybir.InstNoOp(
                        name=f"I-waitnop-{nid}",
                        engine=i.engine,
                        sync_info=mybir.SyncInfo(on_wait=[w], on_update=[]),
                        bass_nofuse=True,
                    ))
                i.sync_info = mybir.SyncInfo(
                    on_wait=waits[-1:], on_update=list(si.on_update or []))
                changed = True
            out.append(i)
        if changed:
            blk.instructions = out


def get_nc(bias=True):
    key = f"nc{bias}"
    if key not in _cache:
        _cache[key] = _build_nc(bias=bias)
    return _cache[key]


def run(inputs_by_core, trace=False, bias=True):
    from concourse.bass_utils import run_bass_kernel_spmd
    nc = get_nc(bias=bias)
    return run_bass_kernel_spmd(nc, inputs_by_core, list(range(NCORES)),
                                trace=trace)


def make_in_maps(x, mu, stem_w, stem_b, head_w, head_b, bn_gamma, bn_beta):
    f16 = np.float16
    x = np.asarray(x, np.float32).reshape(B, C, N)
    common = {
        "mu": np.ascontiguousarray(np.asarray(mu, f16)),
        "ws": np.ascontiguousarray(np.asarray(stem_w, f16)),
        "wst": np.ascontiguousarray(np.asarray(stem_w, np.float32).T
                                    .astype(f16)),
        "hwt": np.ascontiguousarray(np.asarray(head_w, np.float32).T
                                    .astype(f16)),
        "bs": np.ascontiguousarray(
            np.stack([np.asarray(stem_b, f16),
                      np.zeros(C, f16)], axis=1)),
        "gm": np.ascontiguousarray(np.asarray(bn_gamma, np.float32)),
        "bt": np.ascontiguousarray(np.asarray(bn_beta, np.float32)),
        "onec": np.ones(P, f16),
    }
    return [
        {"xh": np.ascontiguousarray(x[i * BPC:(i + 1) * BPC].astype(f16)),
         **common}
        for i in range(NCORES)
    ]


def kernel(x, mu, stem_w, stem_b, head_w, head_b, bn_gamma, bn_beta):
    in_maps = make_in_maps(x, mu, stem_w, stem_b, head_w, head_b,
                           bn_gamma, bn_beta)
    res = run(in_maps, trace=False, bias=True)
    out = np.concatenate(
        [np.asarray(res.results[i]["out"]) for i in range(NCORES)], axis=0)
    return out.reshape(B, C, 64, 64).astype(np.float32)

